# revision 4
# baseline (speedup 1.0000x reference)
import sys

sys.path.insert(0, "/opt/trn_rl_repo")
import numpy as np

import concourse.tile as tile
from concourse import bacc, mybir
from concourse.bass_utils import run_bass_kernel_spmd

F32 = mybir.dt.float32
U32 = mybir.dt.uint32
AF = mybir.ActivationFunctionType
ALU = mybir.AluOpType

N_CORES = 8
B = 131072
D_IN = 256
E = 128
K = 9
TILE = 512
ROWS = B // N_CORES


def _build(rows, act=AF.Lrelu):
    nt = rows // TILE
    nb = nt * (TILE // 128)  # 128-row blocks per core
    nc = bacc.Bacc()

    xA_d = nc.dram_tensor("xA", [D_IN, rows], F32, kind="ExternalInput")
    xB_d = nc.dram_tensor("xB", [D_IN, rows], F32, kind="ExternalInput")
    w1T_d = nc.dram_tensor("w1T", [D_IN, 128], F32, kind="ExternalInput")
    w2T_d = nc.dram_tensor("w2T", [128, 128], F32, kind="ExternalInput")
    w3T_d = nc.dram_tensor("w3T", [128, 128], F32, kind="ExternalInput")
    w4T_d = nc.dram_tensor("w4T", [128, E], F32, kind="ExternalInput")
    wd2T_d = nc.dram_tensor("wd2T", [128, 128], F32, kind="ExternalInput")
    wd3T_d = nc.dram_tensor("wd3T", [128, 128], F32, kind="ExternalInput")
    wd4T_d = nc.dram_tensor("wd4T", [128, 1], F32, kind="ExternalInput")
    g1a_d = nc.dram_tensor("g1a", [K, 128], F32, kind="ExternalInput")
    g1b_d = nc.dram_tensor("g1b", [K, 128], F32, kind="ExternalInput")
    cb2T_d = nc.dram_tensor("cb2T", [E, K], F32, kind="ExternalInput")
    negn_d = nc.dram_tensor("negn", [1, K], F32, kind="ExternalInput")
    ones1_d = nc.dram_tensor("ones1", [1, 128], F32, kind="ExternalInput")
    iden_d = nc.dram_tensor("iden", [128, 128], F32, kind="ExternalInput")
    bias_names = ["be1", "be2", "be3", "be4", "bd1", "bd2", "bd3"]
    bias_d = {n: nc.dram_tensor(n, [128, 1], F32, kind="ExternalInput") for n in bias_names}
    bd4_d = nc.dram_tensor("bd4", [1, 1], F32, kind="ExternalInput")

    resid_d = nc.dram_tensor("resid", [1, rows], F32, kind="ExternalOutput")
    tst_d = {
        "A": nc.dram_tensor("tstA", [128, nb * 8], F32, kind="ExternalOutput"),
        "B": nc.dram_tensor("tstB", [128, nb * 8], F32, kind="ExternalOutput"),
    }
    idx_d = {
        "A": nc.dram_tensor("idxA", [128, nb * 8], U32, kind="ExternalOutput"),
        "B": nc.dram_tensor("idxB", [128, nb * 8], U32, kind="ExternalOutput"),
    }
    s2_d = {
        "A": nc.dram_tensor("s2A", [128, nt], F32, kind="ExternalOutput"),
        "B": nc.dram_tensor("s2B", [128, nt], F32, kind="ExternalOutput"),
    }
    sv_d = {
        "A": nc.dram_tensor("svA", [128, nt], F32, kind="ExternalOutput"),
        "B": nc.dram_tensor("svB", [128, nt], F32, kind="ExternalOutput"),
    }

    with tile.TileContext(nc) as tc:
        with tc.tile_pool(name="cp", bufs=1) as cp, tc.tile_pool(
            name="xp", bufs=2
        ) as xp, tc.tile_pool(name="hp", bufs=2) as hp, tc.tile_pool(
            name="zp", bufs=2
        ) as zp, tc.tile_pool(name="sp", bufs=2) as sp, tc.tile_pool(
            name="pp_h", bufs=2, space="PSUM"
        ) as pp_h, tc.tile_pool(name="pp_sc", bufs=2, space="PSUM") as pp_sc, tc.tile_pool(
            name="pp_oh", bufs=2, space="PSUM"
        ) as pp_oh, tc.tile_pool(name="pp_t", bufs=2, space="PSUM") as pp_t:
            w1a = cp.tile([128, 128], F32, name="w1a")
            w1b = cp.tile([128, 128], F32, name="w1b")
            nc.sync.dma_start(out=w1a[:], in_=w1T_d[0:128, :])
            nc.sync.dma_start(out=w1b[:], in_=w1T_d[128:256, :])
            consts = {}
            for nm, d, shp in [
                ("w2T", w2T_d, [128, 128]),
                ("w3T", w3T_d, [128, 128]),
                ("w4T", w4T_d, [128, E]),
                ("wd2T", wd2T_d, [128, 128]),
                ("wd3T", wd3T_d, [128, 128]),
                ("wd4T", wd4T_d, [128, 1]),
                ("g1a", g1a_d, [K, 128]),
                ("g1b", g1b_d, [K, 128]),
                ("cb2T", cb2T_d, [E, K]),
                ("negn", negn_d, [1, K]),
                ("ones1", ones1_d, [1, 128]),
                ("iden", iden_d, [128, 128]),
                ("bd4", bd4_d, [1, 1]),
            ]:
                t = cp.tile(shp, F32, name=nm)
                nc.sync.dma_start(out=t[:], in_=d[:])
                consts[nm] = t
            for n in bias_names:
                t = cp.tile([128, 1], F32, name=n)
                nc.sync.dma_start(out=t[:], in_=bias_d[n][:])
                consts[n] = t

            resid_st = cp.tile([1, rows], F32, name="resid_st")
            tst_st = {x: cp.tile([128, nb * 8], F32, name="tst" + x) for x in "AB"}
            idx_st = {x: cp.tile([128, nb * 8], U32, name="idx" + x) for x in "AB"}
            s2_st = {x: cp.tile([128, nt], F32, name="s2" + x) for x in "AB"}
            sv_st = {x: cp.tile([128, nt], F32, name="sv" + x) for x in "AB"}

            def encode(tag, xa, xb, i):
                p = pp_h.tile([128, TILE], F32, name="p_mm")
                nc.tensor.matmul(p[:], w1a[:], xa[:], start=True, stop=False)
                nc.tensor.matmul(p[:], w1b[:], xb[:], start=False, stop=True)
                h1 = hp.tile([128, TILE], F32, name="h1")
                nc.scalar.activation(
                    h1[:], p[:], act, bias=consts["be1"][:], scale=1.0, alpha=0.01
                )
                p2 = pp_h.tile([128, TILE], F32, name="p_mm")
                nc.tensor.matmul(p2[:], consts["w2T"][:], h1[:], start=True, stop=True)
                h2 = hp.tile([128, TILE], F32, name="h2")
                nc.scalar.activation(
                    h2[:], p2[:], act, bias=consts["be2"][:], scale=1.0, alpha=0.01
                )
                p3 = pp_h.tile([128, TILE], F32, name="p_mm")
                nc.tensor.matmul(p3[:], consts["w3T"][:], h2[:], start=True, stop=True)
                h3 = hp.tile([128, TILE], F32, name="h3")
                nc.scalar.activation(
                    h3[:], p3[:], act, bias=consts["be3"][:], scale=1.0, alpha=0.01
                )
                p4 = pp_h.tile([128, TILE], F32, name="p_mm")
                nc.tensor.matmul(p4[:], consts["w4T"][:], h3[:], start=True, stop=True)
                z = zp.tile([E, TILE], F32, name="z" + tag)
                nc.scalar.activation(
                    z[:],
                    p4[:],
                    AF.Identity,
                    bias=consts["be4"][:],
                    accum_out=sv_st[tag][:, i : i + 1],
                )
                sq = hp.tile([E, TILE], F32, name="sq")
                nc.scalar.activation(
                    sq[:], z[:], AF.Square, accum_out=s2_st[tag][:, i : i + 1]
                )
                return z

            def vq(tag, z, i):
                oh_fm = zp.tile([K, TILE], F32, name="oh" + tag)
                for c in range(TILE // 128):
                    blk = i * (TILE // 128) + c
                    psc = pp_sc.tile([128, K], F32, name="psc")
                    nc.tensor.matmul(
                        psc[:],
                        z[:, c * 128 : (c + 1) * 128],
                        consts["cb2T"][:],
                        start=True,
                        stop=False,
                    )
                    nc.tensor.matmul(
                        psc[:], consts["ones1"][:], consts["negn"][:], start=False, stop=True
                    )
                    sc = sp.tile([128, K], F32, name="sc")
                    nc.scalar.copy(sc[:], psc[:])
                    tsl = tst_st[tag][:, blk * 8 : (blk + 1) * 8]
                    nc.vector.max(tsl, sc[:])
                    nc.vector.max_index(idx_st[tag][:, blk * 8 : (blk + 1) * 8], tsl, sc[:])
                    oh = sp.tile([128, K], F32, name="oh")
                    nc.vector.tensor_scalar(
                        out=oh[:],
                        in0=sc[:],
                        scalar1=tst_st[tag][:, blk * 8 : blk * 8 + 1],
                        scalar2=None,
                        op0=ALU.is_ge,
                    )
                    pt = pp_oh.tile([K, 128], F32, name="pt")
                    nc.tensor.transpose(pt[:], oh[:], consts["iden"][:])
                    nc.scalar.copy(oh_fm[:, c * 128 : (c + 1) * 128], pt[:])
                return oh_fm

            def decode(name, ohp, ohq):
                p = pp_h.tile([128, TILE], F32, name="p_mm")
                nc.tensor.matmul(p[:], consts["g1a"][:], ohp[:], start=True, stop=False)
                nc.tensor.matmul(p[:], consts["g1b"][:], ohq[:], start=False, stop=True)
                h1 = hp.tile([128, TILE], F32, name="h1")
                nc.scalar.activation(
                    h1[:], p[:], act, bias=consts["bd1"][:], scale=1.0, alpha=0.01
                )
                p2 = pp_h.tile([128, TILE], F32, name="p_mm")
                nc.tensor.matmul(p2[:], consts["wd2T"][:], h1[:], start=True, stop=True)
                h2 = hp.tile([128, TILE], F32, name="h2")
                nc.scalar.activation(
                    h2[:], p2[:], act, bias=consts["bd2"][:], scale=1.0, alpha=0.01
                )
                p3 = pp_h.tile([128, TILE], F32, name="p_mm")
                nc.tensor.matmul(p3[:], consts["wd3T"][:], h2[:], start=True, stop=True)
                h3 = hp.tile([128, TILE], F32, name="h3")
                nc.scalar.activation(
                    h3[:], p3[:], act, bias=consts["bd3"][:], scale=1.0, alpha=0.01
                )
                pt4 = pp_t.tile([1, TILE], F32, name="pt4")
                nc.tensor.matmul(pt4[:], consts["wd4T"][:], h3[:], start=True, stop=True)
                t = sp.tile([1, TILE], F32, name=name)
                nc.scalar.activation(t[:], pt4[:], AF.Tanh, bias=consts["bd4"][:])
                return t

            for i in range(nt):
                xaA = xp.tile([128, TILE], F32, name="xaA")
                xbA = xp.tile([128, TILE], F32, name="xbA")
                xaB = xp.tile([128, TILE], F32, name="xaB")
                xbB = xp.tile([128, TILE], F32, name="xbB")
                sl = slice(i * TILE, (i + 1) * TILE)
                nc.sync.dma_start(out=xaA[:], in_=xA_d[0:128, sl])
                nc.sync.dma_start(out=xbA[:], in_=xA_d[128:256, sl])
                nc.sync.dma_start(out=xaB[:], in_=xB_d[0:128, sl])
                nc.sync.dma_start(out=xbB[:], in_=xB_d[128:256, sl])

                zA = encode("A", xaA, xbA, i)
                zB = encode("B", xaB, xbB, i)
                ohA = vq("A", zA, i)
                ohB = vq("B", zB, i)
                t1 = decode("t1", ohA, ohB)
                t2 = decode("t2", ohB, ohA)
                nc.vector.tensor_sub(resid_st[0:1, sl], t1[:], t2[:])

            nc.sync.dma_start(out=resid_d[:], in_=resid_st[:])
            for x in "AB":
                nc.sync.dma_start(out=tst_d[x][:], in_=tst_st[x][:])
                nc.sync.dma_start(out=idx_d[x][:], in_=idx_st[x][:])
                nc.sync.dma_start(out=s2_d[x][:], in_=s2_st[x][:])
                nc.sync.dma_start(out=sv_d[x][:], in_=sv_st[x][:])

    nc.finalize()
    return nc


def _host_pack(inputs):
    f64 = np.float64
    cb = inputs["codebook"].astype(f64)
    Wd1 = inputs["Wd1"].astype(f64)
    shared = {
        "w1T": np.ascontiguousarray(inputs["We1"].T),
        "w2T": np.ascontiguousarray(inputs["We2"].T),
        "w3T": np.ascontiguousarray(inputs["We3"].T),
        "w4T": np.ascontiguousarray(inputs["We4"].T),
        "wd2T": np.ascontiguousarray(inputs["Wd2"].T),
        "wd3T": np.ascontiguousarray(inputs["Wd3"].T),
        "wd4T": np.ascontiguousarray(inputs["Wd4"].T),
        "g1a": (cb @ Wd1[:, :E].T).astype(np.float32),
        "g1b": (cb @ Wd1[:, E:].T).astype(np.float32),
        "cb2T": np.ascontiguousarray((2.0 * inputs["codebook"]).T),
        "negn": (-(cb**2).sum(1)).astype(np.float32).reshape(1, K),
        "ones1": np.ones((1, 128), np.float32),
        "iden": np.eye(128, dtype=np.float32),
        "be1": inputs["be1"].reshape(128, 1),
        "be2": inputs["be2"].reshape(128, 1),
        "be3": inputs["be3"].reshape(128, 1),
        "be4": inputs["be4"].reshape(128, 1),
        "bd1": inputs["bd1"].reshape(128, 1),
        "bd2": inputs["bd2"].reshape(128, 1),
        "bd3": inputs["bd3"].reshape(128, 1),
        "bd4": inputs["bd4"].reshape(1, 1),
    }
    return {k: np.ascontiguousarray(v, dtype=np.float32) for k, v in shared.items()}


def _gather(results, rows, codebook):
    n = len(results)
    b_eff = n * rows
    nblk = rows // 128
    resid_parts, idx = [], {"A": [], "B": []}
    S2 = {"A": 0.0, "B": 0.0}
    Tst = {"A": 0.0, "B": 0.0}
    Sv = {"A": np.zeros(E), "B": np.zeros(E)}
    for r in results:
        resid_parts.append(r["resid"][0].astype(np.float32) * np.float32(0.5))
        for x in "AB":
            st = r["idx" + x].reshape(128, nblk, 8)[:, :, 0]
            idx[x].append(st.T.reshape(-1).astype(np.int32))
            ts = r["tst" + x].reshape(128, nblk, 8)[:, :, 0]
            Tst[x] += ts.astype(np.float64).sum()
            S2[x] += r["s2" + x].astype(np.float64).sum()
            Sv[x] += r["sv" + x].astype(np.float64).sum(axis=1)
    residual = np.concatenate(resid_parts).reshape(b_eff, 1)
    idx_A = np.concatenate(idx["A"]).reshape(b_eff, 1)
    idx_B = np.concatenate(idx["B"]).reshape(b_eff, 1)
    cb = codebook.astype(np.float64)
    mcb = cb.mean(axis=0)
    den = float(b_eff) * E
    lv = {x: (S2[x] - Tst[x]) / den for x in "AB"}
    lm = {
        x: (S2[x] - 2.0 * (mcb * Sv[x]).sum() + b_eff * (mcb**2).sum()) / den for x in "AB"
    }
    loss_vq = np.float32((lv["A"] + lv["B"]) / 2.0)
    loss_mean = np.float32((lm["A"] + lm["B"]) / 2.0)
    return residual, loss_vq, loss_mean, idx_A, idx_B


_NC_CACHE = {}


def _get_nc(rows):
    if rows not in _NC_CACHE:
        _NC_CACHE[rows] = _build(rows)
    return _NC_CACHE[rows]


def _run(inputs, trace=False):
    nc = _get_nc(ROWS)
    shared = _host_pack(inputs)
    in_maps = []
    for c in range(N_CORES):
        sl = slice(c * ROWS, (c + 1) * ROWS)
        m = dict(shared)
        m["xA"] = np.ascontiguousarray(inputs["comp_A"][sl].T)
        m["xB"] = np.ascontiguousarray(inputs["comp_B"][sl].T)
        in_maps.append(m)
    res = run_bass_kernel_spmd(nc, in_maps, core_ids=list(range(N_CORES)), trace=trace)
    out = _gather(res.results, ROWS, inputs["codebook"])
    return out, res.exec_time_ns


def kernel(**inputs):
    return _run(inputs)[0]


# revision 15
# speedup vs baseline: 79.8900x; 79.8900x over previous
import sys

sys.path.insert(0, "/opt/trn_rl_repo")
import numpy as np

import concourse.tile as tile
from concourse import bacc, mybir
from concourse.bass_utils import run_bass_kernel_spmd

F32 = mybir.dt.float32
F32R = mybir.dt.float32r
U32 = mybir.dt.uint32
AF = mybir.ActivationFunctionType
ALU = mybir.AluOpType

N_CORES = 8
B = 131072
D_IN = 256
E = 128
K = 9
TILE = 512
ROWS = B // N_CORES


def _build(rows, act=AF.Lrelu, fast=True):
    nt = rows // TILE
    nb = nt * (TILE // 128)  # 128-row blocks per core
    nc = bacc.Bacc()
    DEC = F32R if fast else F32

    xA_d = nc.dram_tensor("xA", [D_IN, rows], F32, kind="ExternalInput")
    xB_d = nc.dram_tensor("xB", [D_IN, rows], F32, kind="ExternalInput")
    w1T_d = nc.dram_tensor("w1T", [D_IN, 128], F32, kind="ExternalInput")
    w2T_d = nc.dram_tensor("w2T", [128, 128], F32, kind="ExternalInput")
    w3T_d = nc.dram_tensor("w3T", [128, 128], F32, kind="ExternalInput")
    w4T_d = nc.dram_tensor("w4T", [128, E], F32, kind="ExternalInput")
    wd2T_d = nc.dram_tensor("wd2T", [128, 128], F32, kind="ExternalInput")
    wd3T_d = nc.dram_tensor("wd3T", [128, 128], F32, kind="ExternalInput")
    wd4T_d = nc.dram_tensor("wd4T", [128, 1], F32, kind="ExternalInput")
    g1a_d = nc.dram_tensor("g1a", [K, 128], F32, kind="ExternalInput")
    g1b_d = nc.dram_tensor("g1b", [K, 128], F32, kind="ExternalInput")
    cb2T_d = nc.dram_tensor("cb2T", [E, K], F32, kind="ExternalInput")
    negn_d = nc.dram_tensor("negn", [1, K], F32, kind="ExternalInput")
    ones1_d = nc.dram_tensor("ones1", [1, 128], F32, kind="ExternalInput")
    iden_d = nc.dram_tensor("iden", [128, 128], F32, kind="ExternalInput")
    bias_names = ["be1", "be2", "be3", "be4", "bd1", "bd2", "bd3"]
    bias_d = {n: nc.dram_tensor(n, [128, 1], F32, kind="ExternalInput") for n in bias_names}
    bd4_d = nc.dram_tensor("bd4", [1, 1], F32, kind="ExternalInput")

    resid_d = nc.dram_tensor("resid", [1, rows], F32, kind="ExternalOutput")
    tst_d = {
        "A": nc.dram_tensor("tstA", [128, nb * 8], F32, kind="ExternalOutput"),
        "B": nc.dram_tensor("tstB", [128, nb * 8], F32, kind="ExternalOutput"),
    }
    idx_d = {
        "A": nc.dram_tensor("idxA", [128, nb * 8], U32, kind="ExternalOutput"),
        "B": nc.dram_tensor("idxB", [128, nb * 8], U32, kind="ExternalOutput"),
    }
    s2_d = {
        "A": nc.dram_tensor("s2A", [128, nt], F32, kind="ExternalOutput"),
        "B": nc.dram_tensor("s2B", [128, nt], F32, kind="ExternalOutput"),
    }
    sv_d = {
        "A": nc.dram_tensor("svA", [128, nt], F32, kind="ExternalOutput"),
        "B": nc.dram_tensor("svB", [128, nt], F32, kind="ExternalOutput"),
    }

    with tile.TileContext(nc) as tc:
        with tc.tile_pool(name="cp", bufs=1) as cp, tc.tile_pool(
            name="xp", bufs=2
        ) as xp, tc.tile_pool(name="hp", bufs=2) as hp, tc.tile_pool(
            name="zp", bufs=2
        ) as zp, tc.tile_pool(name="sp", bufs=2) as sp, tc.tile_pool(
            name="pp_h", bufs=2, space="PSUM"
        ) as pp_h, tc.tile_pool(name="pp_sc", bufs=2, space="PSUM") as pp_sc, tc.tile_pool(
            name="pp_oh", bufs=2, space="PSUM"
        ) as pp_oh, tc.tile_pool(name="pp_t", bufs=2, space="PSUM") as pp_t:
            w1a = cp.tile([128, 128], F32, name="w1a")
            w1b = cp.tile([128, 128], F32, name="w1b")
            nc.sync.dma_start(out=w1a[:], in_=w1T_d[0:128, :])
            nc.sync.dma_start(out=w1b[:], in_=w1T_d[128:256, :])
            consts = {}
            for nm, d, shp in [
                ("w2T", w2T_d, [128, 128]),
                ("w3T", w3T_d, [128, 128]),
                ("w4T", w4T_d, [128, E]),
                ("wd2T", wd2T_d, [128, 128]),
                ("wd3T", wd3T_d, [128, 128]),
                ("wd4T", wd4T_d, [128, 1]),
                ("g1a", g1a_d, [K, 128]),
                ("g1b", g1b_d, [K, 128]),
                ("cb2T", cb2T_d, [E, K]),
                ("negn", negn_d, [1, K]),
                ("ones1", ones1_d, [1, 128]),
                ("iden", iden_d, [128, 128]),
                ("bd4", bd4_d, [1, 1]),
            ]:
                t = cp.tile(shp, F32, name=nm)
                nc.sync.dma_start(out=t[:], in_=d[:])
                consts[nm] = t
            if fast:
                for nm, shp in [
                    ("g1a", [K, 128]),
                    ("g1b", [K, 128]),
                    ("wd2T", [128, 128]),
                    ("wd3T", [128, 128]),
                    ("wd4T", [128, 1]),
                    ("iden", [128, 128]),
                ]:
                    tr_ = cp.tile(shp, F32R, name=nm + "r")
                    nc.scalar.copy(tr_[:], consts[nm][:])
                    consts[nm + "r"] = tr_

            def cw(nm):
                return consts[nm + "r"] if fast else consts[nm]

            for n in bias_names:
                t = cp.tile([128, 1], F32, name=n)
                nc.sync.dma_start(out=t[:], in_=bias_d[n][:])
                consts[n] = t

            resid_st = cp.tile([1, rows], F32, name="resid_st")
            tst_st = {x: cp.tile([128, nb * 8], F32, name="tst" + x) for x in "AB"}
            idx_st = {x: cp.tile([128, nb * 8], U32, name="idx" + x) for x in "AB"}
            s2_st = {x: cp.tile([128, nt], F32, name="s2" + x) for x in "AB"}
            sv_st = {x: cp.tile([128, nt], F32, name="sv" + x) for x in "AB"}

            def encode(tag, xa, xb, i):
                p = pp_h.tile([128, TILE], F32, name="p_mm")
                nc.tensor.matmul(p[:], w1a[:], xa[:], start=True, stop=False)
                nc.tensor.matmul(p[:], w1b[:], xb[:], start=False, stop=True)
                h1 = hp.tile([128, TILE], F32, name="h1")
                nc.scalar.activation(
                    h1[:], p[:], act, bias=consts["be1"][:], scale=1.0, alpha=0.01
                )
                p2 = pp_h.tile([128, TILE], F32, name="p_mm")
                nc.tensor.matmul(p2[:], consts["w2T"][:], h1[:], start=True, stop=True)
                h2 = hp.tile([128, TILE], F32, name="h2")
                nc.scalar.activation(
                    h2[:], p2[:], act, bias=consts["be2"][:], scale=1.0, alpha=0.01
                )
                p3 = pp_h.tile([128, TILE], F32, name="p_mm")
                nc.tensor.matmul(p3[:], consts["w3T"][:], h2[:], start=True, stop=True)
                h3 = hp.tile([128, TILE], F32, name="h3")
                nc.scalar.activation(
                    h3[:], p3[:], act, bias=consts["be3"][:], scale=1.0, alpha=0.01
                )
                p4 = pp_h.tile([128, TILE], F32, name="p_mm")
                nc.tensor.matmul(p4[:], consts["w4T"][:], h3[:], start=True, stop=True)
                z = zp.tile([E, TILE], F32, name="z" + tag)
                nc.scalar.activation(
                    z[:],
                    p4[:],
                    AF.Identity,
                    bias=consts["be4"][:],
                    accum_out=sv_st[tag][:, i : i + 1],
                )
                sq = hp.tile([E, TILE], F32, name="sq")
                nc.scalar.activation(
                    sq[:], z[:], AF.Square, accum_out=s2_st[tag][:, i : i + 1]
                )
                return z

            def vq(tag, z, i):
                oh_fm = zp.tile([K, TILE], DEC, name="oh" + tag)
                for c in range(TILE // 128):
                    blk = i * (TILE // 128) + c
                    psc = pp_sc.tile([128, K], F32, name="psc")
                    nc.tensor.matmul(
                        psc[:],
                        z[:, c * 128 : (c + 1) * 128],
                        consts["cb2T"][:],
                        start=True,
                        stop=False,
                    )
                    nc.tensor.matmul(
                        psc[:], consts["ones1"][:], consts["negn"][:], start=False, stop=True
                    )
                    sc = sp.tile([128, K], F32, name="sc")
                    if fast:
                        nc.vector.tensor_copy(sc[:], psc[:])
                    else:
                        nc.scalar.copy(sc[:], psc[:])
                    tsl = tst_st[tag][:, blk * 8 : (blk + 1) * 8]
                    nc.vector.max(tsl, sc[:])
                    nc.vector.max_index(idx_st[tag][:, blk * 8 : (blk + 1) * 8], tsl, sc[:])
                    oh = sp.tile([128, K], DEC, name="oh")
                    nc.vector.tensor_scalar(
                        out=oh[:],
                        in0=sc[:],
                        scalar1=tst_st[tag][:, blk * 8 : blk * 8 + 1],
                        scalar2=None,
                        op0=ALU.is_ge,
                    )
                    pt = pp_oh.tile([K, 128], DEC, name="pt")
                    nc.tensor.transpose(pt[:], oh[:], cw("iden")[:])
                    if fast:
                        nc.vector.tensor_copy(oh_fm[:, c * 128 : (c + 1) * 128], pt[:])
                    else:
                        nc.scalar.copy(oh_fm[:, c * 128 : (c + 1) * 128], pt[:])
                return oh_fm

            def decode(name, ohp, ohq):
                p = pp_h.tile([128, TILE], F32, name="p_mm")
                nc.tensor.matmul(p[:], cw("g1a")[:], ohp[:], start=True, stop=False)
                nc.tensor.matmul(p[:], cw("g1b")[:], ohq[:], start=False, stop=True)
                h1 = hp.tile([128, TILE], DEC, name="hd1")
                nc.scalar.activation(
                    h1[:], p[:], act, bias=consts["bd1"][:], scale=1.0, alpha=0.01
                )
                p2 = pp_h.tile([128, TILE], F32, name="p_mm")
                nc.tensor.matmul(p2[:], cw("wd2T")[:], h1[:], start=True, stop=True)
                h2 = hp.tile([128, TILE], DEC, name="hd2")
                nc.scalar.activation(
                    h2[:], p2[:], act, bias=consts["bd2"][:], scale=1.0, alpha=0.01
                )
                p3 = pp_h.tile([128, TILE], F32, name="p_mm")
                nc.tensor.matmul(p3[:], cw("wd3T")[:], h2[:], start=True, stop=True)
                h3 = hp.tile([128, TILE], DEC, name="hd3")
                nc.scalar.activation(
                    h3[:], p3[:], act, bias=consts["bd3"][:], scale=1.0, alpha=0.01
                )
                pt4 = pp_t.tile([1, TILE], F32, name="pt4")
                nc.tensor.matmul(pt4[:], cw("wd4T")[:], h3[:], start=True, stop=True)
                t = sp.tile([1, TILE], F32, name=name)
                nc.scalar.activation(t[:], pt4[:], AF.Tanh, bias=consts["bd4"][:])
                return t

            for i in range(nt):
                xaA = xp.tile([128, TILE], F32, name="xaA")
                xbA = xp.tile([128, TILE], F32, name="xbA")
                xaB = xp.tile([128, TILE], F32, name="xaB")
                xbB = xp.tile([128, TILE], F32, name="xbB")
                sl = slice(i * TILE, (i + 1) * TILE)
                nc.sync.dma_start(out=xaA[:], in_=xA_d[0:128, sl])
                nc.sync.dma_start(out=xbA[:], in_=xA_d[128:256, sl])
                nc.sync.dma_start(out=xaB[:], in_=xB_d[0:128, sl])
                nc.sync.dma_start(out=xbB[:], in_=xB_d[128:256, sl])

                zA = encode("A", xaA, xbA, i)
                zB = encode("B", xaB, xbB, i)
                ohA = vq("A", zA, i)
                ohB = vq("B", zB, i)
                t1 = decode("t1", ohA, ohB)
                t2 = decode("t2", ohB, ohA)
                nc.vector.tensor_sub(resid_st[0:1, sl], t1[:], t2[:])

            nc.sync.dma_start(out=resid_d[:], in_=resid_st[:])
            for x in "AB":
                nc.sync.dma_start(out=tst_d[x][:], in_=tst_st[x][:])
                nc.sync.dma_start(out=idx_d[x][:], in_=idx_st[x][:])
                nc.sync.dma_start(out=s2_d[x][:], in_=s2_st[x][:])
                nc.sync.dma_start(out=sv_d[x][:], in_=sv_st[x][:])

    nc.finalize()
    return nc


def _host_pack(inputs):
    f64 = np.float64
    cb = inputs["codebook"].astype(f64)
    Wd1 = inputs["Wd1"].astype(f64)
    shared = {
        "w1T": np.ascontiguousarray(inputs["We1"].T),
        "w2T": np.ascontiguousarray(inputs["We2"].T),
        "w3T": np.ascontiguousarray(inputs["We3"].T),
        "w4T": np.ascontiguousarray(inputs["We4"].T),
        "wd2T": np.ascontiguousarray(inputs["Wd2"].T),
        "wd3T": np.ascontiguousarray(inputs["Wd3"].T),
        "wd4T": np.ascontiguousarray(inputs["Wd4"].T),
        "g1a": (cb @ Wd1[:, :E].T).astype(np.float32),
        "g1b": (cb @ Wd1[:, E:].T).astype(np.float32),
        "cb2T": np.ascontiguousarray((2.0 * inputs["codebook"]).T),
        "negn": (-(cb**2).sum(1)).astype(np.float32).reshape(1, K),
        "ones1": np.ones((1, 128), np.float32),
        "iden": np.eye(128, dtype=np.float32),
        "be1": inputs["be1"].reshape(128, 1),
        "be2": inputs["be2"].reshape(128, 1),
        "be3": inputs["be3"].reshape(128, 1),
        "be4": inputs["be4"].reshape(128, 1),
        "bd1": inputs["bd1"].reshape(128, 1),
        "bd2": inputs["bd2"].reshape(128, 1),
        "bd3": inputs["bd3"].reshape(128, 1),
        "bd4": inputs["bd4"].reshape(1, 1),
    }
    return {k: np.ascontiguousarray(v, dtype=np.float32) for k, v in shared.items()}


def _gather(results, rows, codebook):
    n = len(results)
    b_eff = n * rows
    nblk = rows // 128
    resid_parts, idx = [], {"A": [], "B": []}
    S2 = {"A": 0.0, "B": 0.0}
    Tst = {"A": 0.0, "B": 0.0}
    Sv = {"A": np.zeros(E), "B": np.zeros(E)}
    for r in results:
        resid_parts.append(r["resid"][0].astype(np.float32) * np.float32(0.5))
        for x in "AB":
            st = r["idx" + x].reshape(128, nblk, 8)[:, :, 0]
            idx[x].append(st.T.reshape(-1).astype(np.int32))
            ts = r["tst" + x].reshape(128, nblk, 8)[:, :, 0]
            Tst[x] += ts.astype(np.float64).sum()
            S2[x] += r["s2" + x].astype(np.float64).sum()
            Sv[x] += r["sv" + x].astype(np.float64).sum(axis=1)
    residual = np.concatenate(resid_parts).reshape(b_eff, 1)
    idx_A = np.concatenate(idx["A"]).reshape(b_eff, 1)
    idx_B = np.concatenate(idx["B"]).reshape(b_eff, 1)
    cb = codebook.astype(np.float64)
    mcb = cb.mean(axis=0)
    den = float(b_eff) * E
    lv = {x: (S2[x] - Tst[x]) / den for x in "AB"}
    lm = {
        x: (S2[x] - 2.0 * (mcb * Sv[x]).sum() + b_eff * (mcb**2).sum()) / den for x in "AB"
    }
    loss_vq = np.float32((lv["A"] + lv["B"]) / 2.0)
    loss_mean = np.float32((lm["A"] + lm["B"]) / 2.0)
    return residual, loss_vq, loss_mean, idx_A, idx_B


_NC_CACHE = {}


def _get_nc(rows):
    if rows not in _NC_CACHE:
        _NC_CACHE[rows] = _build(rows)
    return _NC_CACHE[rows]


def _run(inputs, trace=False):
    nc = _get_nc(ROWS)
    shared = _host_pack(inputs)
    in_maps = []
    for c in range(N_CORES):
        sl = slice(c * ROWS, (c + 1) * ROWS)
        m = dict(shared)
        m["xA"] = np.ascontiguousarray(inputs["comp_A"][sl].T)
        m["xB"] = np.ascontiguousarray(inputs["comp_B"][sl].T)
        in_maps.append(m)
    res = run_bass_kernel_spmd(nc, in_maps, core_ids=list(range(N_CORES)), trace=trace)
    out = _gather(res.results, ROWS, inputs["codebook"])
    return out, res.exec_time_ns


def kernel(**inputs):
    return _run(inputs)[0]
